# revision 11
# baseline (speedup 1.0000x reference)
"""Trainium2 Bass kernel for nn_Classifier_66116726554918.

BERT-style 6-layer encoder classifier. Strategy: 8-way data-parallel over the
batch (B=16 -> 2 sequences per NeuronCore). Each core runs the full forward on
its 1024 tokens with weights replicated (cast to bf16 host-side for matmuls,
fp32 LayerNorm/softmax statistics). Embedding rows are fetched with dma_gather
(indirect DMA) from HBM. Device returns per-core logits; the tiny final
log-softmax + NLL loss is computed on host from the gathered [16,10] logits.

Self-contained: hardcodes all shapes from the problem spec.
"""
import os
import sys
import numpy as np
from contextlib import ExitStack

sys.path.insert(0, "/opt/trn_rl_repo")

import ml_dtypes
import concourse.bass as bass
import concourse.tile as tile
import concourse.mybir as mybir
from concourse import bacc, library_config
from concourse.bass_utils import run_bass_kernel_spmd
from concourse.masks import make_identity

# model dims
B, S, H, NH, FF, V, LAB, FS = 16, 512, 768, 12, 3072, 30522, 10, 1502
DH = H // NH              # 64
NCORES = 8
BPC = B // NCORES         # 2 batches per core
T = BPC * S               # 1024 tokens per core
P = 128
HT = H // P               # 6 hidden tiles
TT = T // P               # 8 token tiles
FFT = FF // P             # 24 ffn tiles
KG = 6                    # ffn2 contraction group size (in 128-tiles)
CHF = 256                 # ffn token chunk
NL = int(os.environ.get("BERT_L", "6"))
DEBUG = bool(int(os.environ.get("BERT_DEBUG", "0")))

F32 = mybir.dt.float32
BF16 = mybir.dt.bfloat16
I16 = mybir.dt.int16
AF = mybir.ActivationFunctionType
OP = mybir.AluOpType
AX = mybir.AxisListType

_cached = {}


def _nsplit(n):
    out, o = [], 0
    while o < n:
        c = min(512, n - o)
        out.append((o, c))
        o += c
    return out


def build_kernel(nl=NL):
    nc = bacc.Bacc("TRN2", target_bir_lowering=False, debug=False)

    def din(name, shape, dt=F32):
        return nc.dram_tensor(name, shape, dt, kind="ExternalInput").ap()

    word_emb = din("word_emb", [V, H])
    pos_emb = din("pos_emb", [S, H])
    seg_emb_p = din("seg_emb_p", [4, H])
    src_w = din("src_w", [P, T // 16], I16)
    seg_w = din("seg_w", [P, T // 16], I16)
    tok_col_d = din("tok_col", [P, TT])
    seg_col_d = din("seg_col", [P, TT])
    sel_d = din("sel", [FS])
    emb_g_d = din("emb_g", [H])
    emb_b_d = din("emb_b", [H])
    Wq_d = din("Wq", [nl, H, H], BF16)
    Wk_d = din("Wk", [nl, H, H], BF16)
    Wv_d = din("Wv", [nl, H, H], BF16)
    Wo_d = din("Wo", [nl, H, H], BF16)
    bq_d = din("bq", [nl, H])
    bk_d = din("bk", [nl, H])
    bo_d = din("bo_adj", [nl, H])             # bo + bv @ Wo (host-folded)
    Wf1_d = din("Wf1", [nl, FFT, P, H], BF16)  # host-rearranged
    bf1_d = din("bf1", [nl, FF])
    Wf2_d = din("Wf2", [nl, FF, H], BF16)
    bf2_d = din("bf2", [nl, H])
    ln1g_d = din("ln1g", [nl, H])
    ln1b_d = din("ln1b", [nl, H])
    ln2g_d = din("ln2g", [nl, H])
    ln2b_d = din("ln2b", [nl, H])
    Wp1_d = din("Wp1", [H, H], BF16)
    bp1_d = din("bp1", [H])
    Wp2_d = din("Wp2", [H, LAB], BF16)
    bp2_d = din("bp2", [LAB])
    logitsT_d = nc.dram_tensor("logitsT", [LAB, BPC], F32,
                               kind="ExternalOutput").ap()
    dbg_d = None
    if DEBUG:
        dbg_d = nc.dram_tensor("dbg", [nl + 1, T, H], F32,
                               kind="ExternalOutput").ap()

    with tile.TileContext(nc) as tc, ExitStack() as ctx:
        nc.gpsimd.load_library(library_config.mlp)

        singles = ctx.enter_context(tc.tile_pool(name="singles", bufs=1))
        xpool = ctx.enter_context(tc.tile_pool(name="xpool", bufs=9))
        ypool = ctx.enter_context(tc.tile_pool(name="ypool", bufs=4))
        lnwpool = ctx.enter_context(tc.tile_pool(name="lnwpool", bufs=3))
        colpool = ctx.enter_context(tc.tile_pool(name="colpool", bufs=4))
        smpool = ctx.enter_context(tc.tile_pool(name="smpool", bufs=8))
        scratch = ctx.enter_context(tc.tile_pool(name="scratch", bufs=3))

        pp_proj = ctx.enter_context(tc.tile_pool(name="pp_proj", bufs=4, space="PSUM"))
        pp_sc = ctx.enter_context(tc.tile_pool(name="pp_sc", bufs=2, space="PSUM"))
        pp_av = ctx.enter_context(tc.tile_pool(name="pp_av", bufs=1, space="PSUM"))
        pp_bc = ctx.enter_context(tc.tile_pool(name="pp_bc", bufs=1, space="PSUM"))
        pp_tp = pp_sc

        # one-time constants
        ident = singles.tile([P, P], F32)
        make_identity(nc, ident)
        eps_t = singles.tile([P, 1], F32)
        nc.vector.memset(eps_t[:], 1e-5)
        zero_t = singles.tile([P, 1], F32)
        nc.vector.memset(zero_t[:], 0.0)
        ones_row = singles.tile([1, P], F32)
        nc.vector.memset(ones_row[:1, :], 1.0)

        def bcast_load(dram_row, n=H):
            t = lnwpool.tile([P, n], F32, tag="lnw")
            nc.sync.dma_start(t[:], bass.AP(dram_row.tensor, dram_row.offset,
                                            [[0, P], [1, n]]))
            return t

        def col_load(dram_row, ncol, tag="col"):
            t = colpool.tile([P, ncol], F32, tag=tag)
            nc.sync.dma_start(t[:], dram_row.rearrange("(j p) -> p j", p=P))
            return t

        def layernorm(y, g_bc, b_bc, x_out):
            sc = scratch.tile([P, H], F32, tag="lnsc")
            s_sum = smpool.tile([P, 1], F32, tag="s0")
            nc.scalar.activation(sc[:], y[:], AF.Identity, bias=zero_t[:],
                                 accum_out=s_sum[:])
            s_sq = smpool.tile([P, 1], F32, tag="s1")
            nc.scalar.activation(sc[:], y[:], AF.Square, bias=zero_t[:],
                                 accum_out=s_sq[:])
            m = smpool.tile([P, 1], F32, tag="s2")
            nc.vector.tensor_scalar_mul(m[:], s_sum[:], 1.0 / H)
            var = smpool.tile([P, 1], F32, tag="s3")
            msq = smpool.tile([P, 1], F32, tag="s4")
            nc.vector.tensor_mul(msq[:], m[:], m[:])
            nc.vector.tensor_scalar(var[:], s_sq[:], 1.0 / H, None, op0=OP.mult)
            nc.vector.tensor_sub(var[:], var[:], msq[:])
            sd = smpool.tile([P, 1], F32, tag="s5")
            nc.scalar.activation(sd[:], var[:], AF.Sqrt, bias=eps_t[:])
            r = smpool.tile([P, 1], F32, tag="s6")
            nc.vector.reciprocal(r[:], sd[:])
            xh = scratch.tile([P, H], F32, tag="lnsc")
            nc.vector.tensor_scalar(xh[:], y[:], m[:], r[:],
                                    op0=OP.subtract, op1=OP.mult)
            nc.vector.tensor_mul(xh[:], xh[:], g_bc[:])
            nc.vector.tensor_add(x_out[:], xh[:], b_bc[:])

        def transpose_to_xT(x_tiles, xT):
            for t in range(TT):
                for kt in range(HT):
                    pt = pp_tp.tile([P, P], F32, tag="sc")
                    nc.tensor.transpose(pt[:], x_tiles[t][:, kt * P:(kt + 1) * P],
                                        ident[:])
                    nc.scalar.copy(xT[:, kt, t * P:(t + 1) * P], pt[:])

        # ---------------- embedding ----------------
        with tc.tile_pool(name="embpool", bufs=2) as embpool, \
             tc.tile_pool(name="embsingle", bufs=1) as embsingle:
            iota = embsingle.tile([P, FS], F32)
            nc.gpsimd.iota(iota[:], pattern=[[1, FS]], base=0, channel_multiplier=0,
                           allow_small_or_imprecise_dtypes=True)
            selb = embsingle.tile([P, FS], F32)
            nc.sync.dma_start(selb[:], bass.AP(sel_d.tensor, sel_d.offset,
                                               [[0, P], [1, FS]]))
            tok_col = embsingle.tile([P, TT], F32)
            nc.sync.dma_start(tok_col[:], tok_col_d[:])
            emb_g_bc = bcast_load(emb_g_d)
            emb_b_bc = bcast_load(emb_b_d)
            idx_src = embsingle.tile([P, T // 16], I16)
            nc.sync.dma_start(idx_src[:], src_w[:])
            idx_seg = embsingle.tile([P, T // 16], I16)
            nc.sync.dma_start(idx_seg[:], seg_w[:])

            x_tiles = []
            CH = 256
            for ch in range(T // CH):
                gw = embpool.tile([P, CH // P, H], F32, tag="gw")
                nc.gpsimd.dma_gather(
                    gw[:], word_emb[:],
                    idx_src[:, ch * (CH // 16):(ch + 1) * (CH // 16)], CH, CH, H)
                gs = embpool.tile([P, CH // P, H], F32, tag="gs")
                nc.gpsimd.dma_gather(
                    gs[:], seg_emb_p[:],
                    idx_seg[:, ch * (CH // 16):(ch + 1) * (CH // 16)], CH, CH, H)
                for j in range(CH // P):
                    t = ch * (CH // P) + j
                    pos_t = embpool.tile([P, H], F32, tag="pos")
                    nc.sync.dma_start(pos_t[:],
                                      pos_emb[(t % (S // P)) * P:
                                              ((t % (S // P)) + 1) * P, :])
                    y = ypool.tile([P, H], F32, tag="y")
                    nc.vector.tensor_add(y[:], gw[:, j, :], gs[:, j, :])
                    nc.vector.tensor_add(y[:], y[:], pos_t[:])
                    xo = xpool.tile([P, H], F32, tag="x")
                    layernorm(y, emb_g_bc, emb_b_bc, xo)
                    # selection_factor lookup: reduce(onehot(iota==tok)*selb)
                    oh = embpool.tile([P, FS], F32, tag="oh")
                    nc.vector.tensor_scalar(oh[:], iota[:], tok_col[:, t:t + 1],
                                            None, op0=OP.is_equal)
                    nc.vector.tensor_mul(oh[:], oh[:], selb[:])
                    sv = smpool.tile([P, 1], F32, tag="sv")
                    nc.vector.tensor_reduce(sv[:], oh[:], AX.X, OP.add)
                    nc.vector.tensor_scalar_mul(xo[:], xo[:], sv[:])
                    x_tiles.append(xo)

        xtpool = ctx.enter_context(tc.tile_pool(name="xtpool", bufs=2))
        qkpool = ctx.enter_context(tc.tile_pool(name="qkpool", bufs=1))
        vpool = ctx.enter_context(tc.tile_pool(name="vpool", bufs=1))
        ctxpool = ctx.enter_context(tc.tile_pool(name="ctxpool", bufs=1))
        exppool = ctx.enter_context(tc.tile_pool(name="exppool", bufs=6))
        htpool = ctx.enter_context(tc.tile_pool(name="htpool", bufs=1))
        wpool = ctx.enter_context(tc.tile_pool(name="wpool", bufs=2))
        wfpool = ctx.enter_context(tc.tile_pool(name="wfpool", bufs=3))
        wf2pool = ctx.enter_context(tc.tile_pool(name="wf2pool", bufs=2))
        ctpool = ctx.enter_context(tc.tile_pool(name="ctpool", bufs=2))
        recpool = ctx.enter_context(tc.tile_pool(name="recpool", bufs=2))

        seg_col = singles.tile([P, TT], F32)
        nc.sync.dma_start(seg_col[:], seg_col_d[:])
        mask_t = singles.tile([P, TT], F32)
        nc.vector.tensor_scalar(mask_t[:], seg_col[:], 0.0, None, op0=OP.is_gt)
        nc.vector.tensor_scalar(mask_t[:], mask_t[:], 1e9, -1e9,
                                op0=OP.mult, op1=OP.add)

        def dump_dbg(stage, tiles):
            if dbg_d is not None:
                for tm in range(TT):
                    nc.sync.dma_start(dbg_d[stage, tm * P:(tm + 1) * P, :],
                                      tiles[tm][:])

        dump_dbg(0, x_tiles)
        xT = xtpool.tile([P, HT, T], BF16, tag="xT")
        transpose_to_xT(x_tiles, xT)

        # ---------------- layers ----------------
        for l in range(nl):
            def wload(wd, li=l):
                t = wpool.tile([P, HT, H], BF16, tag="w")
                nc.sync.dma_start(
                    t[:], wd[li].rearrange("(kt p) d -> p kt d", p=P))
                return t

            bq_c = col_load(bq_d[l], HT, tag="bqc")
            bk_c = col_load(bk_d[l], HT, tag="bkc")

            Wq_t = wload(Wq_d)
            qT = qkpool.tile([P, HT, T], BF16, tag="qT")
            for dm in range(HT):
                pss = [pp_proj.tile([P, 512], F32, tag="proj", name="pssp")
                       for _ in _nsplit(T)]
                for kt in range(HT):
                    for ni, (no, nn) in enumerate(_nsplit(T)):
                        nc.tensor.matmul(pss[ni][:, :nn],
                                         Wq_t[:, kt, dm * P:(dm + 1) * P],
                                         xT[:, kt, no:no + nn],
                                         start=(kt == 0), stop=(kt == HT - 1))
                for ni, (no, nn) in enumerate(_nsplit(T)):
                    nc.scalar.activation(qT[:, dm, no:no + nn], pss[ni][:, :nn],
                                         AF.Identity, bias=bq_c[:, dm:dm + 1])
            Wk_t = wload(Wk_d)
            kT = qkpool.tile([P, HT, T], BF16, tag="kT")
            for dm in range(HT):
                pss = [pp_proj.tile([P, 512], F32, tag="proj", name="pssp")
                       for _ in _nsplit(T)]
                for kt in range(HT):
                    for ni, (no, nn) in enumerate(_nsplit(T)):
                        nc.tensor.matmul(pss[ni][:, :nn],
                                         Wk_t[:, kt, dm * P:(dm + 1) * P],
                                         xT[:, kt, no:no + nn],
                                         start=(kt == 0), stop=(kt == HT - 1))
                for ni, (no, nn) in enumerate(_nsplit(T)):
                    nc.scalar.activation(kT[:, dm, no:no + nn], pss[ni][:, :nn],
                                         AF.Identity, bias=bk_c[:, dm:dm + 1])
            Wv_t = wload(Wv_d)
            Vp = vpool.tile([P, TT, NH, DH + 1], BF16, tag="Vp")
            nc.vector.memset(Vp[:, :, :, DH:DH + 1], 1.0)
            for tm in range(TT):
                pss = [pp_proj.tile([P, 512], F32, tag="proj", name="pssp")
                       for _ in _nsplit(H)]
                for kt in range(HT):
                    for ni, (no, nn) in enumerate(_nsplit(H)):
                        nc.tensor.matmul(pss[ni][:, :nn],
                                         xT[:, kt, tm * P:(tm + 1) * P],
                                         Wv_t[:, kt, no:no + nn],
                                         start=(kt == 0), stop=(kt == HT - 1))
                for ni, (no, nn) in enumerate(_nsplit(H)):
                    h0 = no // DH
                    nc.scalar.copy(
                        Vp[:, tm, h0:h0 + nn // DH, 0:DH],
                        pss[ni][:, :nn].rearrange("p (h d) -> p h d", d=DH))

            # -- attention
            ctxT = ctxpool.tile([P, HT, T], BF16, tag="ctxT")
            for b in range(BPC):
                for h in range(NH):
                    ht_, po = h // 2, (h % 2) * DH
                    exps = []
                    for kt in range(S // P):
                        ps = pp_sc.tile([P, 512], F32, tag="sc")
                        nc.tensor.matmul(
                            ps[:],
                            kT[po:po + DH, ht_,
                               b * S + kt * P:b * S + (kt + 1) * P],
                            qT[po:po + DH, ht_, b * S:(b + 1) * S],
                            start=True, stop=True)
                        e = exppool.tile([P, 512], BF16, tag="exp")
                        nc.scalar.activation(
                            e[:], ps[:], AF.Exp, scale=float(1.0 / np.sqrt(DH)),
                            bias=mask_t[:, b * (S // P) + kt:
                                        b * (S // P) + kt + 1])
                        exps.append(e)
                    av = pp_av.tile([DH + 1, 512], F32, tag="av")
                    for kt in range(S // P):
                        nc.tensor.matmul(av[:], Vp[:, b * (S // P) + kt, h, :],
                                         exps[kt][:],
                                         start=(kt == 0), stop=(kt == S // P - 1))
                    rec = recpool.tile([1, 512], F32, tag="rec")
                    nc.vector.reciprocal(rec[:1, :], av[DH:DH + 1, :])
                    bc = pp_bc.tile([DH, 512], F32, tag="bc")
                    nc.tensor.matmul(bc[:], ones_row[:1, :DH], rec[:1, :],
                                     start=True, stop=True)
                    ctmp = ctpool.tile([DH, 512], F32, tag="ctmp")
                    nc.vector.tensor_copy(ctmp[:DH, :], av[0:DH, :])
                    nc.vector.tensor_mul(ctxT[po:po + DH, ht_,
                                              b * S:(b + 1) * S],
                                         ctmp[:DH, :], bc[:DH, :])

            # -- attention out proj + residual + LN1
            Wo_t = wload(Wo_d)
            bo_bc = bcast_load(bo_d[l])
            g1_bc = bcast_load(ln1g_d[l])
            b1_bc = bcast_load(ln1b_d[l])
            x_mid = []
            for tm in range(TT):
                y = ypool.tile([P, H], F32, tag="y")
                pss = [pp_proj.tile([P, 512], F32, tag="proj", name="pssp")
                       for _ in _nsplit(H)]
                for kt in range(HT):
                    for ni, (no, nn) in enumerate(_nsplit(H)):
                        nc.tensor.matmul(pss[ni][:, :nn],
                                         ctxT[:, kt, tm * P:(tm + 1) * P],
                                         Wo_t[:, kt, no:no + nn],
                                         start=(kt == 0), stop=(kt == HT - 1))
                for ni, (no, nn) in enumerate(_nsplit(H)):
                    nc.vector.tensor_add(y[:, no:no + nn], pss[ni][:, :nn],
                                         bo_bc[:, no:no + nn])
                nc.vector.tensor_add(y[:], y[:], x_tiles[tm][:])
                xo = xpool.tile([P, H], F32, tag="x")
                layernorm(y, g1_bc, b1_bc, xo)
                x_mid.append(xo)
            xT_mid = xtpool.tile([P, HT, T], BF16, tag="xT")
            transpose_to_xT(x_mid, xT_mid)

            # -- FFN
            bf1_c = col_load(bf1_d[l], FFT, tag="bf1c")
            bf2_bc = bcast_load(bf2_d[l])
            g2_bc = bcast_load(ln2g_d[l])
            b2_bc = bcast_load(ln2b_d[l])
            x_out = []
            for no in range(0, T, CHF):
                nn = CHF
                hT = htpool.tile([P, FFT, CHF], BF16, tag="hT")
                for fm in range(FFT):
                    wf1c = wfpool.tile([P, HT, P], BF16, tag="wf1")
                    nc.sync.dma_start(
                        wf1c[:], Wf1_d[l, fm].rearrange("p (kt j) -> p kt j",
                                                        kt=HT))
                    ps = pp_proj.tile([P, 512], F32, tag="proj")
                    for kt in range(HT):
                        nc.tensor.matmul(ps[:, :nn], wf1c[:, kt, :],
                                         xT_mid[:, kt, no:no + nn],
                                         start=(kt == 0), stop=(kt == HT - 1))
                    nc.scalar.activation(hT[:, fm, :nn], ps[:, :nn],
                                         AF.Gelu_apprx_tanh,
                                         bias=bf1_c[:, fm:fm + 1])
                # FFN2 with grouped Wf2 streaming + SBUF accumulation
                ys = []
                for tm2 in range(nn // P):
                    tm = no // P + tm2
                    y = ypool.tile([P, H], F32, tag="y")
                    nc.vector.tensor_add(y[:], x_mid[tm][:], bf2_bc[:])
                    ys.append(y)
                for g in range(FFT // KG):
                    wf2g = wf2pool.tile([P, KG, H], BF16, tag="wf2")
                    nc.sync.dma_start(
                        wf2g[:], Wf2_d[l].rearrange("(kt p) d -> p kt d", p=P)
                        [:, g * KG:(g + 1) * KG, :])
                    for tm2 in range(nn // P):
                        pss = [pp_proj.tile([P, 512], F32, tag="proj", name="pssp")
                               for _ in _nsplit(H)]
                        for j in range(KG):
                            for hi, (ho, hn) in enumerate(_nsplit(H)):
                                nc.tensor.matmul(
                                    pss[hi][:, :hn],
                                    hT[:, g * KG + j, tm2 * P:(tm2 + 1) * P],
                                    wf2g[:, j, ho:ho + hn],
                                    start=(j == 0), stop=(j == KG - 1))
                        for hi, (ho, hn) in enumerate(_nsplit(H)):
                            nc.vector.tensor_add(ys[tm2][:, ho:ho + hn],
                                                 ys[tm2][:, ho:ho + hn],
                                                 pss[hi][:, :hn])
                for tm2 in range(nn // P):
                    xo = xpool.tile([P, H], F32, tag="x")
                    layernorm(ys[tm2], g2_bc, b2_bc, xo)
                    x_out.append(xo)
            x_tiles = x_mid = x_out
            dump_dbg(l + 1, x_tiles)
            xT = xtpool.tile([P, HT, T], BF16, tag="xT")
            transpose_to_xT(x_tiles, xT)

        # ---------------- pooler + logits ----------------
        xT_f = xT
        Wp1_t = wpool.tile([P, HT, H], BF16, tag="w")
        nc.sync.dma_start(Wp1_t[:], Wp1_d.rearrange("(kt p) d -> p kt d", p=P))
        bp1_c = col_load(bp1_d, HT, tag="bp1c")
        hidT = singles.tile([P, HT, BPC], BF16)
        for hm in range(HT):
            ps = pp_proj.tile([P, 512], F32, tag="proj")
            for kt in range(HT):
                pooled = xT_f[:, kt, :].rearrange("p (a b) -> p a b", a=BPC)[:, :, 0]
                nc.tensor.matmul(ps[:, :BPC], Wp1_t[:, kt, hm * P:(hm + 1) * P],
                                 pooled, start=(kt == 0), stop=(kt == HT - 1))
            nc.scalar.activation(hidT[:, hm, :], ps[:, :BPC], AF.Tanh,
                                 bias=bp1_c[:, hm:hm + 1])
        Wp2_t = singles.tile([P, HT, LAB], BF16)
        nc.sync.dma_start(Wp2_t[:], Wp2_d.rearrange("(kt p) d -> p kt d", p=P))
        bp2_c = singles.tile([LAB, 1], F32)
        nc.sync.dma_start(bp2_c[:LAB, :], bp2_d.rearrange("(j p) -> p j", p=LAB))
        ps = pp_proj.tile([P, 512], F32, tag="proj")
        for kt in range(HT):
            nc.tensor.matmul(ps[:LAB, :BPC], Wp2_t[:, kt, :], hidT[:, kt, :],
                             start=(kt == 0), stop=(kt == HT - 1))
        lg = singles.tile([LAB, BPC], F32)
        nc.scalar.activation(lg[:LAB, :], ps[:LAB, :BPC], AF.Identity,
                             bias=bp2_c[:LAB, :])
        nc.sync.dma_start(logitsT_d[:], lg[:LAB, :])

    nc.compile()
    return nc


def _wrap_idx(idx):
    n = len(idx)
    a = np.zeros((16, n // 16), np.int16)
    a[np.arange(n) % 16, np.arange(n) // 16] = idx.astype(np.int16)
    return np.tile(a, (8, 1))


def _col_layout(x):
    return np.ascontiguousarray(np.asarray(x).reshape(TT, P).T).astype(np.float32)


def prepare_inputs(inputs, nl=NL):
    f32 = lambda a: np.asarray(a, np.float32)
    bf = lambda a: np.ascontiguousarray(np.asarray(a, np.float32)
                                        .astype(ml_dtypes.bfloat16))
    src = np.asarray(inputs["src"])
    seg = np.asarray(inputs["seg"])
    tok = np.asarray(inputs["token_indices_batch"])
    Wo = f32(inputs["Wo"])[:nl]
    bv = f32(inputs["bv"])[:nl]
    bo = f32(inputs["bo"])[:nl]
    bo_adj = bo + np.einsum("lh,lhd->ld", bv, Wo)
    seg_emb_p = np.zeros((4, H), np.float32)
    seg_emb_p[:3] = f32(inputs["seg_emb"])
    shared = dict(
        word_emb=f32(inputs["word_emb"]),
        pos_emb=f32(inputs["pos_emb"]),
        seg_emb_p=seg_emb_p,
        sel=f32(inputs["selection_factor"]),
        emb_g=f32(inputs["emb_ln_g"]), emb_b=f32(inputs["emb_ln_b"]),
        Wq=bf(inputs["Wq"][:nl]), Wk=bf(inputs["Wk"][:nl]),
        Wv=bf(inputs["Wv"][:nl]), Wo=bf(Wo),
        bq=f32(inputs["bq"])[:nl], bk=f32(inputs["bk"])[:nl], bo_adj=bo_adj,
        Wf1=bf(np.asarray(inputs["Wf1"])[:nl].reshape(nl, HT, P, FFT, P)
               .transpose(0, 3, 2, 1, 4).reshape(nl, FFT, P, H)),
        bf1=f32(inputs["bf1"])[:nl],
        Wf2=bf(inputs["Wf2"][:nl]), bf2=f32(inputs["bf2"])[:nl],
        ln1g=f32(inputs["ln1_g"])[:nl], ln1b=f32(inputs["ln1_b"])[:nl],
        ln2g=f32(inputs["ln2_g"])[:nl], ln2b=f32(inputs["ln2_b"])[:nl],
        Wp1=bf(inputs["Wp1"]), bp1=f32(inputs["bp1"]),
        Wp2=bf(inputs["Wp2"]), bp2=f32(inputs["bp2"]),
    )
    per_core = []
    for c in range(NCORES):
        sl = slice(c * BPC, (c + 1) * BPC)
        m = dict(shared)
        m["src_w"] = _wrap_idx(src[sl].reshape(T))
        m["seg_w"] = _wrap_idx(seg[sl].reshape(T))
        m["tok_col"] = _col_layout(tok[sl].reshape(T))
        m["seg_col"] = _col_layout(seg[sl].reshape(T))
        per_core.append(m)
    return per_core


def finish(logits, tgt):
    lm = logits - logits.max(1, keepdims=True)
    logp = lm - np.log(np.exp(lm).sum(1, keepdims=True))
    loss = -logp[np.arange(B), np.asarray(tgt)].mean(dtype=np.float32)
    return np.float32(loss)


def get_nc(nl=NL):
    if "nc" not in _cached:
        _cached["nc"] = build_kernel(nl)
    return _cached["nc"]


def kernel(**inputs):
    nc = get_nc(NL)
    per_core = prepare_inputs(inputs, NL)
    res = None
    last_err = None
    for _attempt in range(3):
        try:
            res = run_bass_kernel_spmd(nc, per_core, core_ids=list(range(NCORES)))
            break
        except Exception as e:  # transient NRT_EXEC_UNIT_UNRECOVERABLE seen
            last_err = e
    if res is None:
        raise last_err
    logits = np.concatenate(
        [np.asarray(res.results[c]["logitsT"], np.float32).T
         for c in range(NCORES)], axis=0)
    loss = finish(logits, inputs["tgt"])
    return loss, logits


# revision 13
# speedup vs baseline: 1.1443x; 1.1443x over previous
"""Trainium2 Bass kernel for nn_Classifier_66116726554918.

BERT-style 6-layer encoder classifier. Strategy: 8-way data-parallel over the
batch (B=16 -> 2 sequences per NeuronCore). Each core runs the full forward on
its 1024 tokens with weights replicated (cast to bf16 host-side for matmuls,
fp32 LayerNorm/softmax statistics). Embedding rows are fetched with dma_gather
(indirect DMA) from HBM. Device returns per-core logits; the tiny final
log-softmax + NLL loss is computed on host from the gathered [16,10] logits.

Self-contained: hardcodes all shapes from the problem spec.
"""
import os
import sys
import numpy as np
from contextlib import ExitStack

sys.path.insert(0, "/opt/trn_rl_repo")

import ml_dtypes
import concourse.bass as bass
import concourse.tile as tile
import concourse.mybir as mybir
from concourse import bacc, library_config
from concourse.bass_utils import run_bass_kernel_spmd
from concourse.masks import make_identity

# model dims
B, S, H, NH, FF, V, LAB, FS = 16, 512, 768, 12, 3072, 30522, 10, 1502
DH = H // NH              # 64
NCORES = 8
BPC = B // NCORES         # 2 batches per core
T = BPC * S               # 1024 tokens per core
P = 128
HT = H // P               # 6 hidden tiles
TT = T // P               # 8 token tiles
FFT = FF // P             # 24 ffn tiles
KG = 6                    # ffn2 contraction group size (in 128-tiles)
CHF = 256                 # ffn token chunk
NL = int(os.environ.get("BERT_L", "6"))
DEBUG = bool(int(os.environ.get("BERT_DEBUG", "0")))

F32 = mybir.dt.float32
BF16 = mybir.dt.bfloat16
I16 = mybir.dt.int16
AF = mybir.ActivationFunctionType
OP = mybir.AluOpType
AX = mybir.AxisListType

_cached = {}


def _nsplit(n):
    out, o = [], 0
    while o < n:
        c = min(512, n - o)
        out.append((o, c))
        o += c
    return out


def build_kernel(nl=NL):
    nc = bacc.Bacc("TRN2", target_bir_lowering=False, debug=False)

    def din(name, shape, dt=F32):
        return nc.dram_tensor(name, shape, dt, kind="ExternalInput").ap()

    word_emb = din("word_emb", [V, H])
    pos_emb = din("pos_emb", [S, H])
    seg_emb_p = din("seg_emb_p", [4, H])
    src_w = din("src_w", [P, T // 16], I16)
    seg_w = din("seg_w", [P, T // 16], I16)
    tok_col_d = din("tok_col", [P, TT])
    seg_col_d = din("seg_col", [P, TT])
    sel_d = din("sel", [FS])
    emb_g_d = din("emb_g", [H])
    emb_b_d = din("emb_b", [H])
    Wq_d = din("Wq", [nl, H, H], BF16)
    Wk_d = din("Wk", [nl, H, H], BF16)
    Wv_d = din("Wv", [nl, H, H], BF16)
    Wo_d = din("Wo", [nl, H, H], BF16)
    bq_d = din("bq", [nl, H])
    bk_d = din("bk", [nl, H])
    bo_d = din("bo_adj", [nl, H])             # bo + bv @ Wo (host-folded)
    Wf1_d = din("Wf1", [nl, FFT, P, H], BF16)  # host-rearranged
    bf1_d = din("bf1", [nl, FF])
    Wf2_d = din("Wf2", [nl, FF, H], BF16)
    bf2_d = din("bf2", [nl, H])
    ln1g_d = din("ln1g", [nl, H])
    ln1b_d = din("ln1b", [nl, H])
    ln2g_d = din("ln2g", [nl, H])
    ln2b_d = din("ln2b", [nl, H])
    Wp1_d = din("Wp1", [H, H], BF16)
    bp1_d = din("bp1", [H])
    Wp2_d = din("Wp2", [H, LAB], BF16)
    bp2_d = din("bp2", [LAB])
    logitsT_d = nc.dram_tensor("logitsT", [LAB, BPC], F32,
                               kind="ExternalOutput").ap()
    dbg_d = None
    if DEBUG:
        dbg_d = nc.dram_tensor("dbg", [nl + 1, T, H], F32,
                               kind="ExternalOutput").ap()

    with tile.TileContext(nc) as tc, ExitStack() as ctx:
        nc.gpsimd.load_library(library_config.mlp)

        singles = ctx.enter_context(tc.tile_pool(name="singles", bufs=1))
        xpool = ctx.enter_context(tc.tile_pool(name="xpool", bufs=9))
        ypool = ctx.enter_context(tc.tile_pool(name="ypool", bufs=4))
        lnwpool = ctx.enter_context(tc.tile_pool(name="lnwpool", bufs=3))
        colpool = ctx.enter_context(tc.tile_pool(name="colpool", bufs=4))
        smpool = ctx.enter_context(tc.tile_pool(name="smpool", bufs=8))
        scratch = ctx.enter_context(tc.tile_pool(name="scratch", bufs=3))

        pp_proj = ctx.enter_context(tc.tile_pool(name="pp_proj", bufs=4, space="PSUM"))
        pp_sc = ctx.enter_context(tc.tile_pool(name="pp_sc", bufs=2, space="PSUM"))
        pp_av = ctx.enter_context(tc.tile_pool(name="pp_av", bufs=1, space="PSUM"))
        pp_bc = ctx.enter_context(tc.tile_pool(name="pp_bc", bufs=1, space="PSUM"))
        pp_tp = pp_sc

        # one-time constants
        ident = singles.tile([P, P], F32)
        make_identity(nc, ident)
        eps_t = singles.tile([P, 1], F32)
        nc.vector.memset(eps_t[:], 1e-5)
        zero_t = singles.tile([P, 1], F32)
        nc.vector.memset(zero_t[:], 0.0)
        ones_row = singles.tile([1, P], F32)
        nc.vector.memset(ones_row[:1, :], 1.0)

        def bcast_load(dram_row, n=H):
            t = lnwpool.tile([P, n], F32, tag="lnw")
            nc.sync.dma_start(t[:], bass.AP(dram_row.tensor, dram_row.offset,
                                            [[0, P], [1, n]]))
            return t

        def col_load(dram_row, ncol, tag="col"):
            t = colpool.tile([P, ncol], F32, tag=tag)
            nc.sync.dma_start(t[:], dram_row.rearrange("(j p) -> p j", p=P))
            return t

        def layernorm(y, g_bc, b_bc, x_out):
            sc = scratch.tile([P, H], F32, tag="lnsc")
            s_sum = smpool.tile([P, 1], F32, tag="s0")
            nc.scalar.activation(sc[:], y[:], AF.Identity, bias=zero_t[:],
                                 accum_out=s_sum[:])
            s_sq = smpool.tile([P, 1], F32, tag="s1")
            nc.scalar.activation(sc[:], y[:], AF.Square, bias=zero_t[:],
                                 accum_out=s_sq[:])
            m = smpool.tile([P, 1], F32, tag="s2")
            nc.vector.tensor_scalar_mul(m[:], s_sum[:], 1.0 / H)
            var = smpool.tile([P, 1], F32, tag="s3")
            msq = smpool.tile([P, 1], F32, tag="s4")
            nc.vector.tensor_mul(msq[:], m[:], m[:])
            nc.vector.tensor_scalar(var[:], s_sq[:], 1.0 / H, None, op0=OP.mult)
            nc.vector.tensor_sub(var[:], var[:], msq[:])
            sd = smpool.tile([P, 1], F32, tag="s5")
            nc.scalar.activation(sd[:], var[:], AF.Sqrt, bias=eps_t[:])
            r = smpool.tile([P, 1], F32, tag="s6")
            nc.vector.reciprocal(r[:], sd[:])
            xh = scratch.tile([P, H], F32, tag="lnsc")
            nc.vector.tensor_scalar(xh[:], y[:], m[:], r[:],
                                    op0=OP.subtract, op1=OP.mult)
            nc.vector.tensor_mul(xh[:], xh[:], g_bc[:])
            nc.vector.tensor_add(x_out[:], xh[:], b_bc[:])

        def transpose_to_xT(x_tiles, xT):
            for t in range(TT):
                for kt in range(HT):
                    pt = pp_tp.tile([P, P], F32, tag="sc")
                    nc.tensor.transpose(pt[:], x_tiles[t][:, kt * P:(kt + 1) * P],
                                        ident[:])
                    nc.scalar.copy(xT[:, kt, t * P:(t + 1) * P], pt[:])

        # ---------------- embedding ----------------
        with tc.tile_pool(name="embpool", bufs=2) as embpool, \
             tc.tile_pool(name="embsingle", bufs=1) as embsingle:
            iota = embsingle.tile([P, FS], F32)
            nc.gpsimd.iota(iota[:], pattern=[[1, FS]], base=0, channel_multiplier=0,
                           allow_small_or_imprecise_dtypes=True)
            selb = embsingle.tile([P, FS], F32)
            nc.sync.dma_start(selb[:], bass.AP(sel_d.tensor, sel_d.offset,
                                               [[0, P], [1, FS]]))
            tok_col = embsingle.tile([P, TT], F32)
            nc.sync.dma_start(tok_col[:], tok_col_d[:])
            emb_g_bc = bcast_load(emb_g_d)
            emb_b_bc = bcast_load(emb_b_d)
            idx_src = embsingle.tile([P, T // 16], I16)
            nc.sync.dma_start(idx_src[:], src_w[:])
            idx_seg = embsingle.tile([P, T // 16], I16)
            nc.sync.dma_start(idx_seg[:], seg_w[:])

            x_tiles = []
            CH = 256
            for ch in range(T // CH):
                gw = embpool.tile([P, CH // P, H], F32, tag="gw")
                nc.gpsimd.dma_gather(
                    gw[:], word_emb[:],
                    idx_src[:, ch * (CH // 16):(ch + 1) * (CH // 16)], CH, CH, H)
                gs = embpool.tile([P, CH // P, H], F32, tag="gs")
                nc.gpsimd.dma_gather(
                    gs[:], seg_emb_p[:],
                    idx_seg[:, ch * (CH // 16):(ch + 1) * (CH // 16)], CH, CH, H)
                for j in range(CH // P):
                    t = ch * (CH // P) + j
                    pos_t = embpool.tile([P, H], F32, tag="pos")
                    nc.sync.dma_start(pos_t[:],
                                      pos_emb[(t % (S // P)) * P:
                                              ((t % (S // P)) + 1) * P, :])
                    y = ypool.tile([P, H], F32, tag="y")
                    nc.vector.tensor_add(y[:], gw[:, j, :], gs[:, j, :])
                    nc.vector.tensor_add(y[:], y[:], pos_t[:])
                    xo = xpool.tile([P, H], F32, tag="x")
                    layernorm(y, emb_g_bc, emb_b_bc, xo)
                    # selection_factor lookup: reduce(onehot(iota==tok)*selb)
                    oh = embpool.tile([P, FS], F32, tag="oh")
                    nc.vector.tensor_scalar(oh[:], iota[:], tok_col[:, t:t + 1],
                                            None, op0=OP.is_equal)
                    nc.vector.tensor_mul(oh[:], oh[:], selb[:])
                    sv = smpool.tile([P, 1], F32, tag="sv")
                    nc.vector.tensor_reduce(sv[:], oh[:], AX.X, OP.add)
                    nc.vector.tensor_scalar_mul(xo[:], xo[:], sv[:])
                    x_tiles.append(xo)

        xtpool = ctx.enter_context(tc.tile_pool(name="xtpool", bufs=2))
        qkpool = ctx.enter_context(tc.tile_pool(name="qkpool", bufs=1))
        vpool = ctx.enter_context(tc.tile_pool(name="vpool", bufs=1))
        ctxpool = ctx.enter_context(tc.tile_pool(name="ctxpool", bufs=1))
        exppool = ctx.enter_context(tc.tile_pool(name="exppool", bufs=6))
        htpool = ctx.enter_context(tc.tile_pool(name="htpool", bufs=1))
        wpool = ctx.enter_context(tc.tile_pool(name="wpool", bufs=2))
        wfpool = ctx.enter_context(tc.tile_pool(name="wfpool", bufs=3))
        wf2pool = ctx.enter_context(tc.tile_pool(name="wf2pool", bufs=2))
        ctpool = ctx.enter_context(tc.tile_pool(name="ctpool", bufs=2))
        recpool = ctx.enter_context(tc.tile_pool(name="recpool", bufs=2))

        seg_col = singles.tile([P, TT], F32)
        nc.sync.dma_start(seg_col[:], seg_col_d[:])
        mask_t = singles.tile([P, TT], F32)
        nc.vector.tensor_scalar(mask_t[:], seg_col[:], 0.0, None, op0=OP.is_gt)
        nc.vector.tensor_scalar(mask_t[:], mask_t[:], 1e9, -1e9,
                                op0=OP.mult, op1=OP.add)

        def dump_dbg(stage, tiles):
            if dbg_d is not None:
                for tm in range(TT):
                    nc.sync.dma_start(dbg_d[stage, tm * P:(tm + 1) * P, :],
                                      tiles[tm][:])

        dump_dbg(0, x_tiles)
        xT = xtpool.tile([P, HT, T], BF16, tag="xT")
        transpose_to_xT(x_tiles, xT)

        # ---------------- layers ----------------
        for l in range(nl):
            def wload(wd, li=l):
                t = wpool.tile([P, HT, H], BF16, tag="w")
                nc.sync.dma_start(
                    t[:], wd[li].rearrange("(kt p) d -> p kt d", p=P))
                return t

            bq_c = col_load(bq_d[l], HT, tag="bqc")
            bk_c = col_load(bk_d[l], HT, tag="bkc")

            Wq_t = wload(Wq_d)
            qT = qkpool.tile([P, HT, T], BF16, tag="qT")
            for dm in range(HT):
                pss = [pp_proj.tile([P, 512], F32, tag="proj", name="pssp")
                       for _ in _nsplit(T)]
                for kt in range(HT):
                    for ni, (no, nn) in enumerate(_nsplit(T)):
                        nc.tensor.matmul(pss[ni][:, :nn],
                                         Wq_t[:, kt, dm * P:(dm + 1) * P],
                                         xT[:, kt, no:no + nn],
                                         start=(kt == 0), stop=(kt == HT - 1))
                for ni, (no, nn) in enumerate(_nsplit(T)):
                    nc.scalar.activation(qT[:, dm, no:no + nn], pss[ni][:, :nn],
                                         AF.Identity, bias=bq_c[:, dm:dm + 1])
            Wk_t = wload(Wk_d)
            kT = qkpool.tile([P, HT, T], BF16, tag="kT")
            for dm in range(HT):
                pss = [pp_proj.tile([P, 512], F32, tag="proj", name="pssp")
                       for _ in _nsplit(T)]
                for kt in range(HT):
                    for ni, (no, nn) in enumerate(_nsplit(T)):
                        nc.tensor.matmul(pss[ni][:, :nn],
                                         Wk_t[:, kt, dm * P:(dm + 1) * P],
                                         xT[:, kt, no:no + nn],
                                         start=(kt == 0), stop=(kt == HT - 1))
                for ni, (no, nn) in enumerate(_nsplit(T)):
                    nc.scalar.activation(kT[:, dm, no:no + nn], pss[ni][:, :nn],
                                         AF.Identity, bias=bk_c[:, dm:dm + 1])
            Wv_t = wload(Wv_d)
            Vp = vpool.tile([P, TT, NH, DH + 1], BF16, tag="Vp")
            nc.vector.memset(Vp[:, :, :, DH:DH + 1], 1.0)
            for tm in range(TT):
                pss = [pp_proj.tile([P, 512], F32, tag="proj", name="pssp")
                       for _ in _nsplit(H)]
                for kt in range(HT):
                    for ni, (no, nn) in enumerate(_nsplit(H)):
                        nc.tensor.matmul(pss[ni][:, :nn],
                                         xT[:, kt, tm * P:(tm + 1) * P],
                                         Wv_t[:, kt, no:no + nn],
                                         start=(kt == 0), stop=(kt == HT - 1))
                for ni, (no, nn) in enumerate(_nsplit(H)):
                    h0 = no // DH
                    nc.scalar.copy(
                        Vp[:, tm, h0:h0 + nn // DH, 0:DH],
                        pss[ni][:, :nn].rearrange("p (h d) -> p h d", d=DH))

            # -- attention
            ctxT = ctxpool.tile([P, HT, T], BF16, tag="ctxT")
            for b in range(BPC):
                for h in range(NH):
                    ht_, po = h // 2, (h % 2) * DH
                    exps = []
                    for kt in range(S // P):
                        ps = pp_sc.tile([P, 512], F32, tag="sc")
                        nc.tensor.matmul(
                            ps[:],
                            kT[po:po + DH, ht_,
                               b * S + kt * P:b * S + (kt + 1) * P],
                            qT[po:po + DH, ht_, b * S:(b + 1) * S],
                            start=True, stop=True)
                        e = exppool.tile([P, 512], BF16, tag="exp")
                        nc.scalar.activation(
                            e[:], ps[:], AF.Exp, scale=float(1.0 / np.sqrt(DH)),
                            bias=mask_t[:, b * (S // P) + kt:
                                        b * (S // P) + kt + 1])
                        exps.append(e)
                    av = pp_av.tile([DH + 1, 512], F32, tag="av")
                    for kt in range(S // P):
                        nc.tensor.matmul(av[:], Vp[:, b * (S // P) + kt, h, :],
                                         exps[kt][:],
                                         start=(kt == 0), stop=(kt == S // P - 1))
                    rec = recpool.tile([1, 512], F32, tag="rec")
                    nc.vector.reciprocal(rec[:1, :], av[DH:DH + 1, :])
                    bc = pp_bc.tile([DH, 512], F32, tag="bc")
                    nc.tensor.matmul(bc[:], ones_row[:1, :DH], rec[:1, :],
                                     start=True, stop=True)
                    ctmp = ctpool.tile([DH, 512], F32, tag="ctmp")
                    nc.vector.tensor_copy(ctmp[:DH, :], av[0:DH, :])
                    nc.vector.tensor_mul(ctxT[po:po + DH, ht_,
                                              b * S:(b + 1) * S],
                                         ctmp[:DH, :], bc[:DH, :])

            # -- attention out proj + residual + LN1
            Wo_t = wload(Wo_d)
            bo_bc = bcast_load(bo_d[l])
            g1_bc = bcast_load(ln1g_d[l])
            b1_bc = bcast_load(ln1b_d[l])
            x_mid = []
            for tm in range(TT):
                y = ypool.tile([P, H], F32, tag="y")
                pss = [pp_proj.tile([P, 512], F32, tag="proj", name="pssp")
                       for _ in _nsplit(H)]
                for kt in range(HT):
                    for ni, (no, nn) in enumerate(_nsplit(H)):
                        nc.tensor.matmul(pss[ni][:, :nn],
                                         ctxT[:, kt, tm * P:(tm + 1) * P],
                                         Wo_t[:, kt, no:no + nn],
                                         start=(kt == 0), stop=(kt == HT - 1))
                for ni, (no, nn) in enumerate(_nsplit(H)):
                    nc.vector.tensor_add(y[:, no:no + nn], pss[ni][:, :nn],
                                         bo_bc[:, no:no + nn])
                nc.vector.tensor_add(y[:], y[:], x_tiles[tm][:])
                xo = xpool.tile([P, H], F32, tag="x")
                layernorm(y, g1_bc, b1_bc, xo)
                x_mid.append(xo)
            xT_mid = xtpool.tile([P, HT, T], BF16, tag="xT")
            transpose_to_xT(x_mid, xT_mid)

            # -- FFN
            bf1_c = col_load(bf1_d[l], FFT, tag="bf1c")
            bf2_bc = bcast_load(bf2_d[l])
            g2_bc = bcast_load(ln2g_d[l])
            b2_bc = bcast_load(ln2b_d[l])
            x_out = []
            for no in range(0, T, CHF):
                nn = CHF
                hT = htpool.tile([P, FFT, CHF], BF16, tag="hT")
                for fm in range(FFT):
                    wf1c = wfpool.tile([P, HT, P], BF16, tag="wf1")
                    nc.sync.dma_start(
                        wf1c[:], Wf1_d[l, fm].rearrange("p (kt j) -> p kt j",
                                                        kt=HT))
                    ps = pp_proj.tile([P, 512], F32, tag="proj")
                    for kt in range(HT):
                        nc.tensor.matmul(ps[:, :nn], wf1c[:, kt, :],
                                         xT_mid[:, kt, no:no + nn],
                                         start=(kt == 0), stop=(kt == HT - 1))
                    nc.scalar.activation(hT[:, fm, :nn], ps[:, :nn],
                                         AF.Gelu_apprx_tanh,
                                         bias=bf1_c[:, fm:fm + 1])
                # FFN2 with grouped Wf2 streaming + SBUF accumulation
                ys = []
                for tm2 in range(nn // P):
                    tm = no // P + tm2
                    y = ypool.tile([P, H], F32, tag="y")
                    nc.vector.tensor_add(y[:], x_mid[tm][:], bf2_bc[:])
                    ys.append(y)
                for g in range(FFT // KG):
                    wf2g = wf2pool.tile([P, KG, H], BF16, tag="wf2")
                    nc.sync.dma_start(
                        wf2g[:], Wf2_d[l].rearrange("(kt p) d -> p kt d", p=P)
                        [:, g * KG:(g + 1) * KG, :])
                    for tm2 in range(nn // P):
                        pss = [pp_proj.tile([P, 512], F32, tag="proj", name="pssp")
                               for _ in _nsplit(H)]
                        for j in range(KG):
                            for hi, (ho, hn) in enumerate(_nsplit(H)):
                                nc.tensor.matmul(
                                    pss[hi][:, :hn],
                                    hT[:, g * KG + j, tm2 * P:(tm2 + 1) * P],
                                    wf2g[:, j, ho:ho + hn],
                                    start=(j == 0), stop=(j == KG - 1))
                        for hi, (ho, hn) in enumerate(_nsplit(H)):
                            nc.vector.tensor_add(ys[tm2][:, ho:ho + hn],
                                                 ys[tm2][:, ho:ho + hn],
                                                 pss[hi][:, :hn])
                for tm2 in range(nn // P):
                    xo = xpool.tile([P, H], F32, tag="x")
                    layernorm(ys[tm2], g2_bc, b2_bc, xo)
                    x_out.append(xo)
            x_tiles = x_mid = x_out
            dump_dbg(l + 1, x_tiles)
            xT = xtpool.tile([P, HT, T], BF16, tag="xT")
            transpose_to_xT(x_tiles, xT)

        # ---------------- pooler + logits ----------------
        xT_f = xT
        Wp1_t = wpool.tile([P, HT, H], BF16, tag="w")
        nc.sync.dma_start(Wp1_t[:], Wp1_d.rearrange("(kt p) d -> p kt d", p=P))
        bp1_c = col_load(bp1_d, HT, tag="bp1c")
        hidT = singles.tile([P, HT, BPC], BF16)
        for hm in range(HT):
            ps = pp_proj.tile([P, 512], F32, tag="proj")
            for kt in range(HT):
                pooled = xT_f[:, kt, :].rearrange("p (a b) -> p a b", a=BPC)[:, :, 0]
                nc.tensor.matmul(ps[:, :BPC], Wp1_t[:, kt, hm * P:(hm + 1) * P],
                                 pooled, start=(kt == 0), stop=(kt == HT - 1))
            nc.scalar.activation(hidT[:, hm, :], ps[:, :BPC], AF.Tanh,
                                 bias=bp1_c[:, hm:hm + 1])
        Wp2_t = singles.tile([P, HT, LAB], BF16)
        nc.sync.dma_start(Wp2_t[:], Wp2_d.rearrange("(kt p) d -> p kt d", p=P))
        bp2_c = singles.tile([LAB, 1], F32)
        nc.sync.dma_start(bp2_c[:LAB, :], bp2_d.rearrange("(j p) -> p j", p=LAB))
        ps = pp_proj.tile([P, 512], F32, tag="proj")
        for kt in range(HT):
            nc.tensor.matmul(ps[:LAB, :BPC], Wp2_t[:, kt, :], hidT[:, kt, :],
                             start=(kt == 0), stop=(kt == HT - 1))
        lg = singles.tile([LAB, BPC], F32)
        nc.scalar.activation(lg[:LAB, :], ps[:LAB, :BPC], AF.Identity,
                             bias=bp2_c[:LAB, :])
        nc.sync.dma_start(logitsT_d[:], lg[:LAB, :])

    nc.compile()
    return nc


def _wrap_idx(idx):
    n = len(idx)
    a = np.zeros((16, n // 16), np.int16)
    a[np.arange(n) % 16, np.arange(n) // 16] = idx.astype(np.int16)
    return np.tile(a, (8, 1))


def _col_layout(x):
    return np.ascontiguousarray(np.asarray(x).reshape(TT, P).T).astype(np.float32)


def prepare_inputs(inputs, nl=NL):
    f32 = lambda a: np.asarray(a, np.float32)
    bf = lambda a: np.ascontiguousarray(np.asarray(a, np.float32)
                                        .astype(ml_dtypes.bfloat16))
    src = np.asarray(inputs["src"])
    seg = np.asarray(inputs["seg"])
    tok = np.asarray(inputs["token_indices_batch"])
    Wo = f32(inputs["Wo"])[:nl]
    bv = f32(inputs["bv"])[:nl]
    bo = f32(inputs["bo"])[:nl]
    bo_adj = bo + np.einsum("lh,lhd->ld", bv, Wo)
    seg_emb_p = np.zeros((4, H), np.float32)
    seg_emb_p[:3] = f32(inputs["seg_emb"])
    shared = dict(
        word_emb=f32(inputs["word_emb"]),
        pos_emb=f32(inputs["pos_emb"]),
        seg_emb_p=seg_emb_p,
        sel=f32(inputs["selection_factor"]),
        emb_g=f32(inputs["emb_ln_g"]), emb_b=f32(inputs["emb_ln_b"]),
        Wq=bf(inputs["Wq"][:nl]), Wk=bf(inputs["Wk"][:nl]),
        Wv=bf(inputs["Wv"][:nl]), Wo=bf(Wo),
        bq=f32(inputs["bq"])[:nl], bk=f32(inputs["bk"])[:nl], bo_adj=bo_adj,
        Wf1=bf(np.asarray(inputs["Wf1"])[:nl].reshape(nl, HT, P, FFT, P)
               .transpose(0, 3, 2, 1, 4).reshape(nl, FFT, P, H)),
        bf1=f32(inputs["bf1"])[:nl],
        Wf2=bf(inputs["Wf2"][:nl]), bf2=f32(inputs["bf2"])[:nl],
        ln1g=f32(inputs["ln1_g"])[:nl], ln1b=f32(inputs["ln1_b"])[:nl],
        ln2g=f32(inputs["ln2_g"])[:nl], ln2b=f32(inputs["ln2_b"])[:nl],
        Wp1=bf(inputs["Wp1"]), bp1=f32(inputs["bp1"]),
        Wp2=bf(inputs["Wp2"]), bp2=f32(inputs["bp2"]),
    )
    per_core = []
    for c in range(NCORES):
        sl = slice(c * BPC, (c + 1) * BPC)
        m = dict(shared)
        m["src_w"] = _wrap_idx(src[sl].reshape(T))
        m["seg_w"] = _wrap_idx(seg[sl].reshape(T))
        m["tok_col"] = _col_layout(tok[sl].reshape(T))
        m["seg_col"] = _col_layout(seg[sl].reshape(T))
        per_core.append(m)
    return per_core


def finish(logits, tgt):
    lm = logits - logits.max(1, keepdims=True)
    logp = lm - np.log(np.exp(lm).sum(1, keepdims=True))
    loss = -logp[np.arange(B), np.asarray(tgt)].mean(dtype=np.float32)
    return np.float32(loss)


def get_nc(nl=NL):
    if "nc" not in _cached:
        _cached["nc"] = build_kernel(nl)
    return _cached["nc"]


def kernel(**inputs):
    nc = get_nc(NL)
    per_core = prepare_inputs(inputs, NL)
    res = None
    last_err = None
    for _attempt in range(3):
        try:
            res = run_bass_kernel_spmd(nc, per_core, core_ids=list(range(NCORES)))
            break
        except Exception as e:  # transient NRT_EXEC_UNIT_UNRECOVERABLE seen
            last_err = e
    if res is None:
        raise last_err
    logits = np.concatenate(
        [np.asarray(res.results[c]["logitsT"], np.float32).T
         for c in range(NCORES)], axis=0)
    loss = finish(logits, inputs["tgt"])
    return loss, logits


# revision 14
# speedup vs baseline: 1.1879x; 1.0381x over previous
"""Trainium2 Bass kernel for nn_Classifier_66116726554918.

BERT-style 6-layer encoder classifier. Strategy: 8-way data-parallel over the
batch (B=16 -> 2 sequences per NeuronCore). Each core runs the full forward on
its 1024 tokens with weights replicated (cast to bf16 host-side for matmuls,
fp32 LayerNorm/softmax statistics). Embedding rows are fetched with dma_gather
(indirect DMA) from HBM. Device returns per-core logits; the tiny final
log-softmax + NLL loss is computed on host from the gathered [16,10] logits.

Self-contained: hardcodes all shapes from the problem spec.
"""
import os
import sys
import numpy as np
from contextlib import ExitStack

sys.path.insert(0, "/opt/trn_rl_repo")

import ml_dtypes
import concourse.bass as bass
import concourse.tile as tile
import concourse.mybir as mybir
from concourse import bacc, library_config
from concourse.bass_utils import run_bass_kernel_spmd
from concourse.masks import make_identity

# model dims
B, S, H, NH, FF, V, LAB, FS = 16, 512, 768, 12, 3072, 30522, 10, 1502
DH = H // NH              # 64
NCORES = 8
BPC = B // NCORES         # 2 batches per core
T = BPC * S               # 1024 tokens per core
P = 128
HT = H // P               # 6 hidden tiles
TT = T // P               # 8 token tiles
FFT = FF // P             # 24 ffn tiles
KG = 6                    # ffn2 contraction group size (in 128-tiles)
CHF = 256                 # ffn token chunk
NL = int(os.environ.get("BERT_L", "6"))
DEBUG = bool(int(os.environ.get("BERT_DEBUG", "0")))

F32 = mybir.dt.float32
BF16 = mybir.dt.bfloat16
I16 = mybir.dt.int16
AF = mybir.ActivationFunctionType
OP = mybir.AluOpType
AX = mybir.AxisListType

_cached = {}


def _nsplit(n):
    out, o = [], 0
    while o < n:
        c = min(512, n - o)
        out.append((o, c))
        o += c
    return out


def build_kernel(nl=NL):
    nc = bacc.Bacc("TRN2", target_bir_lowering=False, debug=False)

    def din(name, shape, dt=F32):
        return nc.dram_tensor(name, shape, dt, kind="ExternalInput").ap()

    word_emb = din("word_emb", [V, H])
    pos_emb = din("pos_emb", [S, H])
    seg_emb_p = din("seg_emb_p", [4, H])
    src_w = din("src_w", [P, T // 16], I16)
    seg_w = din("seg_w", [P, T // 16], I16)
    tok_col_d = din("tok_col", [P, TT])
    seg_col_d = din("seg_col", [P, TT])
    sel_d = din("sel", [FS])
    emb_g_d = din("emb_g", [H])
    emb_b_d = din("emb_b", [H])
    Wq_d = din("Wq", [nl, H, H], BF16)
    Wk_d = din("Wk", [nl, H, H], BF16)
    Wv_d = din("Wv", [nl, H, H], BF16)
    Wo_d = din("Wo", [nl, H, H], BF16)
    bq_d = din("bq", [nl, H])
    bk_d = din("bk", [nl, H])
    bo_d = din("bo_adj", [nl, H])             # bo + bv @ Wo (host-folded)
    Wf1_d = din("Wf1", [nl, FFT, P, H], BF16)  # host-rearranged
    bf1_d = din("bf1", [nl, FF])
    Wf2_d = din("Wf2", [nl, FF, H], BF16)
    bf2_d = din("bf2", [nl, H])
    ln1g_d = din("ln1g", [nl, H])
    ln1b_d = din("ln1b", [nl, H])
    ln2g_d = din("ln2g", [nl, H])
    ln2b_d = din("ln2b", [nl, H])
    Wp1_d = din("Wp1", [H, H], BF16)
    bp1_d = din("bp1", [H])
    Wp2_d = din("Wp2", [H, LAB], BF16)
    bp2_d = din("bp2", [LAB])
    logitsT_d = nc.dram_tensor("logitsT", [LAB, BPC], F32,
                               kind="ExternalOutput").ap()
    dbg_d = None
    if DEBUG:
        dbg_d = nc.dram_tensor("dbg", [nl + 1, T, H], F32,
                               kind="ExternalOutput").ap()

    with tile.TileContext(nc) as tc, ExitStack() as ctx:
        nc.gpsimd.load_library(library_config.mlp)

        singles = ctx.enter_context(tc.tile_pool(name="singles", bufs=1))
        xpool = ctx.enter_context(tc.tile_pool(name="xpool", bufs=9))
        ypool = ctx.enter_context(tc.tile_pool(name="ypool", bufs=4))
        lnwpool = ctx.enter_context(tc.tile_pool(name="lnwpool", bufs=3))
        colpool = ctx.enter_context(tc.tile_pool(name="colpool", bufs=4))
        smpool = ctx.enter_context(tc.tile_pool(name="smpool", bufs=8))
        scratch = ctx.enter_context(tc.tile_pool(name="scratch", bufs=3))

        pp_proj = ctx.enter_context(tc.tile_pool(name="pp_proj", bufs=4, space="PSUM"))
        pp_sc = ctx.enter_context(tc.tile_pool(name="pp_sc", bufs=2, space="PSUM"))
        pp_av = ctx.enter_context(tc.tile_pool(name="pp_av", bufs=1, space="PSUM"))
        pp_bc = ctx.enter_context(tc.tile_pool(name="pp_bc", bufs=1, space="PSUM"))
        pp_tp = pp_sc

        # one-time constants
        ident = singles.tile([P, P], F32)
        make_identity(nc, ident)
        eps_t = singles.tile([P, 1], F32)
        nc.vector.memset(eps_t[:], 1e-5)
        zero_t = singles.tile([P, 1], F32)
        nc.vector.memset(zero_t[:], 0.0)
        ones_row = singles.tile([1, P], F32)
        nc.vector.memset(ones_row[:1, :], 1.0)

        def bcast_load(dram_row, n=H):
            t = lnwpool.tile([P, n], F32, tag="lnw")
            nc.sync.dma_start(t[:], bass.AP(dram_row.tensor, dram_row.offset,
                                            [[0, P], [1, n]]))
            return t

        def col_load(dram_row, ncol, tag="col"):
            t = colpool.tile([P, ncol], F32, tag=tag)
            nc.sync.dma_start(t[:], dram_row.rearrange("(j p) -> p j", p=P))
            return t

        def layernorm(y, g_bc, b_bc, x_out):
            sc = scratch.tile([P, H], F32, tag="lnsc")
            s_sum = smpool.tile([P, 1], F32, tag="s0")
            nc.scalar.activation(sc[:], y[:], AF.Identity, bias=zero_t[:],
                                 accum_out=s_sum[:])
            s_sq = smpool.tile([P, 1], F32, tag="s1")
            nc.scalar.activation(sc[:], y[:], AF.Square, bias=zero_t[:],
                                 accum_out=s_sq[:])
            m = smpool.tile([P, 1], F32, tag="s2")
            nc.vector.tensor_scalar_mul(m[:], s_sum[:], 1.0 / H)
            var = smpool.tile([P, 1], F32, tag="s3")
            msq = smpool.tile([P, 1], F32, tag="s4")
            nc.vector.tensor_mul(msq[:], m[:], m[:])
            nc.vector.tensor_scalar(var[:], s_sq[:], 1.0 / H, None, op0=OP.mult)
            nc.vector.tensor_sub(var[:], var[:], msq[:])
            sd = smpool.tile([P, 1], F32, tag="s5")
            nc.scalar.activation(sd[:], var[:], AF.Sqrt, bias=eps_t[:])
            r = smpool.tile([P, 1], F32, tag="s6")
            nc.vector.reciprocal(r[:], sd[:])
            xh = scratch.tile([P, H], F32, tag="lnsc")
            nc.vector.tensor_scalar(xh[:], y[:], m[:], r[:],
                                    op0=OP.subtract, op1=OP.mult)
            nc.vector.tensor_mul(xh[:], xh[:], g_bc[:])
            nc.vector.tensor_add(x_out[:], xh[:], b_bc[:])

        def transpose_to_xT(x_tiles, xT):
            for t in range(TT):
                for kt in range(HT):
                    pt = pp_tp.tile([P, P], F32, tag="sc")
                    nc.tensor.transpose(pt[:], x_tiles[t][:, kt * P:(kt + 1) * P],
                                        ident[:])
                    nc.scalar.copy(xT[:, kt, t * P:(t + 1) * P], pt[:])

        # ---------------- embedding ----------------
        with tc.tile_pool(name="embpool", bufs=2) as embpool, \
             tc.tile_pool(name="embsingle", bufs=1) as embsingle:
            iota = embsingle.tile([P, FS], F32)
            nc.gpsimd.iota(iota[:], pattern=[[1, FS]], base=0, channel_multiplier=0,
                           allow_small_or_imprecise_dtypes=True)
            selb = embsingle.tile([P, FS], F32)
            nc.sync.dma_start(selb[:], bass.AP(sel_d.tensor, sel_d.offset,
                                               [[0, P], [1, FS]]))
            tok_col = embsingle.tile([P, TT], F32)
            nc.sync.dma_start(tok_col[:], tok_col_d[:])
            emb_g_bc = bcast_load(emb_g_d)
            emb_b_bc = bcast_load(emb_b_d)
            idx_src = embsingle.tile([P, T // 16], I16)
            nc.sync.dma_start(idx_src[:], src_w[:])
            idx_seg = embsingle.tile([P, T // 16], I16)
            nc.sync.dma_start(idx_seg[:], seg_w[:])

            x_tiles = []
            CH = 256
            for ch in range(T // CH):
                gw = embpool.tile([P, CH // P, H], F32, tag="gw")
                nc.gpsimd.dma_gather(
                    gw[:], word_emb[:],
                    idx_src[:, ch * (CH // 16):(ch + 1) * (CH // 16)], CH, CH, H)
                gs = embpool.tile([P, CH // P, H], F32, tag="gs")
                nc.gpsimd.dma_gather(
                    gs[:], seg_emb_p[:],
                    idx_seg[:, ch * (CH // 16):(ch + 1) * (CH // 16)], CH, CH, H)
                for j in range(CH // P):
                    t = ch * (CH // P) + j
                    pos_t = embpool.tile([P, H], F32, tag="pos")
                    nc.sync.dma_start(pos_t[:],
                                      pos_emb[(t % (S // P)) * P:
                                              ((t % (S // P)) + 1) * P, :])
                    y = ypool.tile([P, H], F32, tag="y")
                    nc.vector.tensor_add(y[:], gw[:, j, :], gs[:, j, :])
                    nc.vector.tensor_add(y[:], y[:], pos_t[:])
                    xo = xpool.tile([P, H], F32, tag="x")
                    layernorm(y, emb_g_bc, emb_b_bc, xo)
                    # selection_factor lookup: reduce(onehot(iota==tok)*selb)
                    oh = embpool.tile([P, FS], F32, tag="oh")
                    nc.vector.tensor_scalar(oh[:], iota[:], tok_col[:, t:t + 1],
                                            None, op0=OP.is_equal)
                    nc.vector.tensor_mul(oh[:], oh[:], selb[:])
                    sv = smpool.tile([P, 1], F32, tag="sv")
                    nc.vector.tensor_reduce(sv[:], oh[:], AX.X, OP.add)
                    nc.vector.tensor_scalar_mul(xo[:], xo[:], sv[:])
                    x_tiles.append(xo)

        xtpool = ctx.enter_context(tc.tile_pool(name="xtpool", bufs=2))
        qkpool = ctx.enter_context(tc.tile_pool(name="qkpool", bufs=1))
        vpool = ctx.enter_context(tc.tile_pool(name="vpool", bufs=1))
        ctxpool = ctx.enter_context(tc.tile_pool(name="ctxpool", bufs=1))
        exppool = ctx.enter_context(tc.tile_pool(name="exppool", bufs=4))
        htpool = ctx.enter_context(tc.tile_pool(name="htpool", bufs=2))
        wpool = ctx.enter_context(tc.tile_pool(name="wpool", bufs=1))
        wfpool = ctx.enter_context(tc.tile_pool(name="wfpool", bufs=3))
        wf2pool = ctx.enter_context(tc.tile_pool(name="wf2pool", bufs=2))
        ctpool = ctx.enter_context(tc.tile_pool(name="ctpool", bufs=2))
        recpool = ctx.enter_context(tc.tile_pool(name="recpool", bufs=1))

        seg_col = singles.tile([P, TT], F32)
        nc.sync.dma_start(seg_col[:], seg_col_d[:])
        mask_t = singles.tile([P, TT], F32)
        nc.vector.tensor_scalar(mask_t[:], seg_col[:], 0.0, None, op0=OP.is_gt)
        nc.vector.tensor_scalar(mask_t[:], mask_t[:], 1e9, -1e9,
                                op0=OP.mult, op1=OP.add)

        def dump_dbg(stage, tiles):
            if dbg_d is not None:
                for tm in range(TT):
                    nc.sync.dma_start(dbg_d[stage, tm * P:(tm + 1) * P, :],
                                      tiles[tm][:])

        dump_dbg(0, x_tiles)
        xT = xtpool.tile([P, HT, T], BF16, tag="xT")
        transpose_to_xT(x_tiles, xT)

        # ---------------- layers ----------------
        for l in range(nl):
            def wload(wd, li=l):
                t = wpool.tile([P, HT, H], BF16, tag="w")
                nc.sync.dma_start(
                    t[:], wd[li].rearrange("(kt p) d -> p kt d", p=P))
                return t

            bq_c = col_load(bq_d[l], HT, tag="bqc")
            bk_c = col_load(bk_d[l], HT, tag="bkc")

            Wq_t = wload(Wq_d)
            qT = qkpool.tile([P, HT, T], BF16, tag="qT")
            for dm in range(HT):
                pss = [pp_proj.tile([P, 512], F32, tag="proj", name="pssp")
                       for _ in _nsplit(T)]
                for kt in range(HT):
                    for ni, (no, nn) in enumerate(_nsplit(T)):
                        nc.tensor.matmul(pss[ni][:, :nn],
                                         Wq_t[:, kt, dm * P:(dm + 1) * P],
                                         xT[:, kt, no:no + nn],
                                         start=(kt == 0), stop=(kt == HT - 1))
                for ni, (no, nn) in enumerate(_nsplit(T)):
                    nc.scalar.activation(qT[:, dm, no:no + nn], pss[ni][:, :nn],
                                         AF.Identity, bias=bq_c[:, dm:dm + 1])
            Wk_t = wload(Wk_d)
            kT = qkpool.tile([P, HT, T], BF16, tag="kT")
            for dm in range(HT):
                pss = [pp_proj.tile([P, 512], F32, tag="proj", name="pssp")
                       for _ in _nsplit(T)]
                for kt in range(HT):
                    for ni, (no, nn) in enumerate(_nsplit(T)):
                        nc.tensor.matmul(pss[ni][:, :nn],
                                         Wk_t[:, kt, dm * P:(dm + 1) * P],
                                         xT[:, kt, no:no + nn],
                                         start=(kt == 0), stop=(kt == HT - 1))
                for ni, (no, nn) in enumerate(_nsplit(T)):
                    nc.scalar.activation(kT[:, dm, no:no + nn], pss[ni][:, :nn],
                                         AF.Identity, bias=bk_c[:, dm:dm + 1])
            Wv_t = wload(Wv_d)
            Vp = vpool.tile([P, TT, NH, DH + 1], BF16, tag="Vp")
            nc.vector.memset(Vp[:, :, :, DH:DH + 1], 1.0)
            for tm in range(TT):
                pss = [pp_proj.tile([P, 512], F32, tag="proj", name="pssp")
                       for _ in _nsplit(H)]
                for kt in range(HT):
                    for ni, (no, nn) in enumerate(_nsplit(H)):
                        nc.tensor.matmul(pss[ni][:, :nn],
                                         xT[:, kt, tm * P:(tm + 1) * P],
                                         Wv_t[:, kt, no:no + nn],
                                         start=(kt == 0), stop=(kt == HT - 1))
                for ni, (no, nn) in enumerate(_nsplit(H)):
                    h0 = no // DH
                    nc.scalar.copy(
                        Vp[:, tm, h0:h0 + nn // DH, 0:DH],
                        pss[ni][:, :nn].rearrange("p (h d) -> p h d", d=DH))

            # -- attention
            ctxT = ctxpool.tile([P, HT, T], BF16, tag="ctxT")
            for b in range(BPC):
                for h in range(NH):
                    ht_, po = h // 2, (h % 2) * DH
                    exps = []
                    for kt in range(S // P):
                        ps = pp_sc.tile([P, 512], F32, tag="sc")
                        nc.tensor.matmul(
                            ps[:],
                            kT[po:po + DH, ht_,
                               b * S + kt * P:b * S + (kt + 1) * P],
                            qT[po:po + DH, ht_, b * S:(b + 1) * S],
                            start=True, stop=True)
                        e = exppool.tile([P, 512], BF16, tag="exp")
                        nc.scalar.activation(
                            e[:], ps[:], AF.Exp, scale=float(1.0 / np.sqrt(DH)),
                            bias=mask_t[:, b * (S // P) + kt:
                                        b * (S // P) + kt + 1])
                        exps.append(e)
                    av = pp_av.tile([DH + 1, 512], F32, tag="av")
                    for kt in range(S // P):
                        nc.tensor.matmul(av[:], Vp[:, b * (S // P) + kt, h, :],
                                         exps[kt][:],
                                         start=(kt == 0), stop=(kt == S // P - 1))
                    rec = recpool.tile([1, 512], F32, tag="rec")
                    nc.vector.reciprocal(rec[:1, :], av[DH:DH + 1, :])
                    bc = pp_bc.tile([DH, 512], F32, tag="bc")
                    nc.tensor.matmul(bc[:], ones_row[:1, :DH], rec[:1, :],
                                     start=True, stop=True)
                    ctmp = ctpool.tile([DH, 512], F32, tag="ctmp")
                    nc.vector.tensor_copy(ctmp[:DH, :], av[0:DH, :])
                    nc.vector.tensor_mul(ctxT[po:po + DH, ht_,
                                              b * S:(b + 1) * S],
                                         ctmp[:DH, :], bc[:DH, :])

            # -- attention out proj + residual + LN1
            Wo_t = wload(Wo_d)
            bo_bc = bcast_load(bo_d[l])
            g1_bc = bcast_load(ln1g_d[l])
            b1_bc = bcast_load(ln1b_d[l])
            x_mid = []
            for tm in range(TT):
                y = ypool.tile([P, H], F32, tag="y")
                pss = [pp_proj.tile([P, 512], F32, tag="proj", name="pssp")
                       for _ in _nsplit(H)]
                for kt in range(HT):
                    for ni, (no, nn) in enumerate(_nsplit(H)):
                        nc.tensor.matmul(pss[ni][:, :nn],
                                         ctxT[:, kt, tm * P:(tm + 1) * P],
                                         Wo_t[:, kt, no:no + nn],
                                         start=(kt == 0), stop=(kt == HT - 1))
                for ni, (no, nn) in enumerate(_nsplit(H)):
                    nc.vector.tensor_add(y[:, no:no + nn], pss[ni][:, :nn],
                                         bo_bc[:, no:no + nn])
                nc.vector.tensor_add(y[:], y[:], x_tiles[tm][:])
                xo = xpool.tile([P, H], F32, tag="x")
                layernorm(y, g1_bc, b1_bc, xo)
                x_mid.append(xo)
            xT_mid = xtpool.tile([P, HT, T], BF16, tag="xT")
            transpose_to_xT(x_mid, xT_mid)

            # -- FFN
            bf1_c = col_load(bf1_d[l], FFT, tag="bf1c")
            bf2_bc = bcast_load(bf2_d[l])
            g2_bc = bcast_load(ln2g_d[l])
            b2_bc = bcast_load(ln2b_d[l])
            x_out = []
            NCP = T // CHF // 2          # chunk pairs
            for chp in range(NCP):
                # FFN1 for a PAIR of token chunks: each Wf1 load serves both
                hTs = [htpool.tile([P, FFT, CHF], BF16, tag="hT", name="hTt")
                       for _ in range(2)]
                for fm in range(FFT):
                    wf1c = wfpool.tile([P, HT, P], BF16, tag="wf1")
                    nc.sync.dma_start(
                        wf1c[:], Wf1_d[l, fm].rearrange("p (kt j) -> p kt j",
                                                        kt=HT))
                    pss = [pp_proj.tile([P, 512], F32, tag="proj", name="pssf")
                           for _ in range(2)]
                    for kt in range(HT):
                        for ci in range(2):
                            no = (chp * 2 + ci) * CHF
                            nc.tensor.matmul(pss[ci][:, :CHF], wf1c[:, kt, :],
                                             xT_mid[:, kt, no:no + CHF],
                                             start=(kt == 0),
                                             stop=(kt == HT - 1))
                    for ci in range(2):
                        nc.scalar.activation(hTs[ci][:, fm, :CHF],
                                             pss[ci][:, :CHF],
                                             AF.Gelu_apprx_tanh,
                                             bias=bf1_c[:, fm:fm + 1])
                # FFN2: Wf2 group loads hoisted over both chunks
                ys = []                   # [ci][tm2]
                for ci in range(2):
                    row = []
                    for tm2 in range(CHF // P):
                        tm = (chp * 2 + ci) * (CHF // P) + tm2
                        y = ypool.tile([P, H], F32, tag="y")
                        nc.vector.tensor_add(y[:], x_mid[tm][:], bf2_bc[:])
                        row.append(y)
                    ys.append(row)
                for g in range(FFT // KG):
                    wf2g = wf2pool.tile([P, KG, H], BF16, tag="wf2")
                    nc.sync.dma_start(
                        wf2g[:], Wf2_d[l].rearrange("(kt p) d -> p kt d", p=P)
                        [:, g * KG:(g + 1) * KG, :])
                    for ci in range(2):
                        for tm2 in range(CHF // P):
                            pss = [pp_proj.tile([P, 512], F32, tag="proj",
                                                name="pssp")
                                   for _ in _nsplit(H)]
                            for j in range(KG):
                                for hi, (ho, hn) in enumerate(_nsplit(H)):
                                    nc.tensor.matmul(
                                        pss[hi][:, :hn],
                                        hTs[ci][:, g * KG + j,
                                                tm2 * P:(tm2 + 1) * P],
                                        wf2g[:, j, ho:ho + hn],
                                        start=(j == 0), stop=(j == KG - 1))
                            for hi, (ho, hn) in enumerate(_nsplit(H)):
                                nc.vector.tensor_add(
                                    ys[ci][tm2][:, ho:ho + hn],
                                    ys[ci][tm2][:, ho:ho + hn],
                                    pss[hi][:, :hn])
                for ci in range(2):
                    for tm2 in range(CHF // P):
                        xo = xpool.tile([P, H], F32, tag="x")
                        layernorm(ys[ci][tm2], g2_bc, b2_bc, xo)
                        x_out.append(xo)
            x_tiles = x_mid = x_out
            dump_dbg(l + 1, x_tiles)
            xT = xtpool.tile([P, HT, T], BF16, tag="xT")
            transpose_to_xT(x_tiles, xT)

        # ---------------- pooler + logits ----------------
        xT_f = xT
        Wp1_t = wpool.tile([P, HT, H], BF16, tag="w")
        nc.sync.dma_start(Wp1_t[:], Wp1_d.rearrange("(kt p) d -> p kt d", p=P))
        bp1_c = col_load(bp1_d, HT, tag="bp1c")
        hidT = singles.tile([P, HT, BPC], BF16)
        for hm in range(HT):
            ps = pp_proj.tile([P, 512], F32, tag="proj")
            for kt in range(HT):
                pooled = xT_f[:, kt, :].rearrange("p (a b) -> p a b", a=BPC)[:, :, 0]
                nc.tensor.matmul(ps[:, :BPC], Wp1_t[:, kt, hm * P:(hm + 1) * P],
                                 pooled, start=(kt == 0), stop=(kt == HT - 1))
            nc.scalar.activation(hidT[:, hm, :], ps[:, :BPC], AF.Tanh,
                                 bias=bp1_c[:, hm:hm + 1])
        Wp2_t = singles.tile([P, HT, LAB], BF16)
        nc.sync.dma_start(Wp2_t[:], Wp2_d.rearrange("(kt p) d -> p kt d", p=P))
        bp2_c = singles.tile([LAB, 1], F32)
        nc.sync.dma_start(bp2_c[:LAB, :], bp2_d.rearrange("(j p) -> p j", p=LAB))
        ps = pp_proj.tile([P, 512], F32, tag="proj")
        for kt in range(HT):
            nc.tensor.matmul(ps[:LAB, :BPC], Wp2_t[:, kt, :], hidT[:, kt, :],
                             start=(kt == 0), stop=(kt == HT - 1))
        lg = singles.tile([LAB, BPC], F32)
        nc.scalar.activation(lg[:LAB, :], ps[:LAB, :BPC], AF.Identity,
                             bias=bp2_c[:LAB, :])
        nc.sync.dma_start(logitsT_d[:], lg[:LAB, :])

    nc.compile()
    return nc


def _wrap_idx(idx):
    n = len(idx)
    a = np.zeros((16, n // 16), np.int16)
    a[np.arange(n) % 16, np.arange(n) // 16] = idx.astype(np.int16)
    return np.tile(a, (8, 1))


def _col_layout(x):
    return np.ascontiguousarray(np.asarray(x).reshape(TT, P).T).astype(np.float32)


def prepare_inputs(inputs, nl=NL):
    f32 = lambda a: np.asarray(a, np.float32)
    bf = lambda a: np.ascontiguousarray(np.asarray(a, np.float32)
                                        .astype(ml_dtypes.bfloat16))
    src = np.asarray(inputs["src"])
    seg = np.asarray(inputs["seg"])
    tok = np.asarray(inputs["token_indices_batch"])
    Wo = f32(inputs["Wo"])[:nl]
    bv = f32(inputs["bv"])[:nl]
    bo = f32(inputs["bo"])[:nl]
    bo_adj = bo + np.einsum("lh,lhd->ld", bv, Wo)
    seg_emb_p = np.zeros((4, H), np.float32)
    seg_emb_p[:3] = f32(inputs["seg_emb"])
    shared = dict(
        word_emb=f32(inputs["word_emb"]),
        pos_emb=f32(inputs["pos_emb"]),
        seg_emb_p=seg_emb_p,
        sel=f32(inputs["selection_factor"]),
        emb_g=f32(inputs["emb_ln_g"]), emb_b=f32(inputs["emb_ln_b"]),
        Wq=bf(inputs["Wq"][:nl]), Wk=bf(inputs["Wk"][:nl]),
        Wv=bf(inputs["Wv"][:nl]), Wo=bf(Wo),
        bq=f32(inputs["bq"])[:nl], bk=f32(inputs["bk"])[:nl], bo_adj=bo_adj,
        Wf1=bf(np.asarray(inputs["Wf1"])[:nl].reshape(nl, HT, P, FFT, P)
               .transpose(0, 3, 2, 1, 4).reshape(nl, FFT, P, H)),
        bf1=f32(inputs["bf1"])[:nl],
        Wf2=bf(inputs["Wf2"][:nl]), bf2=f32(inputs["bf2"])[:nl],
        ln1g=f32(inputs["ln1_g"])[:nl], ln1b=f32(inputs["ln1_b"])[:nl],
        ln2g=f32(inputs["ln2_g"])[:nl], ln2b=f32(inputs["ln2_b"])[:nl],
        Wp1=bf(inputs["Wp1"]), bp1=f32(inputs["bp1"]),
        Wp2=bf(inputs["Wp2"]), bp2=f32(inputs["bp2"]),
    )
    per_core = []
    for c in range(NCORES):
        sl = slice(c * BPC, (c + 1) * BPC)
        m = dict(shared)
        m["src_w"] = _wrap_idx(src[sl].reshape(T))
        m["seg_w"] = _wrap_idx(seg[sl].reshape(T))
        m["tok_col"] = _col_layout(tok[sl].reshape(T))
        m["seg_col"] = _col_layout(seg[sl].reshape(T))
        per_core.append(m)
    return per_core


def finish(logits, tgt):
    lm = logits - logits.max(1, keepdims=True)
    logp = lm - np.log(np.exp(lm).sum(1, keepdims=True))
    loss = -logp[np.arange(B), np.asarray(tgt)].mean(dtype=np.float32)
    return np.float32(loss)


def get_nc(nl=NL):
    if "nc" not in _cached:
        _cached["nc"] = build_kernel(nl)
    return _cached["nc"]


def kernel(**inputs):
    nc = get_nc(NL)
    per_core = prepare_inputs(inputs, NL)
    res = None
    last_err = None
    for _attempt in range(3):
        try:
            res = run_bass_kernel_spmd(nc, per_core, core_ids=list(range(NCORES)))
            break
        except Exception as e:  # transient NRT_EXEC_UNIT_UNRECOVERABLE seen
            last_err = e
    if res is None:
        raise last_err
    logits = np.concatenate(
        [np.asarray(res.results[c]["logitsT"], np.float32).T
         for c in range(NCORES)], axis=0)
    loss = finish(logits, inputs["tgt"])
    return loss, logits
